# revision 46
# baseline (speedup 1.0000x reference)
"""GPT self-attention layer (B=2, S=2048, D=1024, H=16, hd=64) on 8 TRN2 cores.

Sharding: data-parallel over batch (2) x tensor-parallel over heads (4 groups
of 4 heads). Core c handles batch b=c//4, head group g=c%4.

v2: fp8e4m3 DoubleRow matmuls for QKV / scores / ctx (4x / 2x / 4x PE rate),
bf16 out-projection and bf16 AllToAll payloads, softmax reciprocal on DVE,
receiver-side batch select on DVE (replaces gpsimd dma_gather).

Per-core pipeline:
  1. xt = x[b].T arrives fp8 (host cast); wq/wk/wv fp8 prescaled by 64 (fp8
     subnormal dodge), descaled for free in the ACT bias op / DVE V-store.
  2. QT/KT = W.T @ x.T via DoubleRow (dc pairs), stored fp8 with a zeroed
     second k-subtile slab so the scores matmul can run DoubleRow with
     contraction 64. V = x @ Wv stored fp8 with interleaved ones column
     (softmax denominator rides along in ctx psum row 64).
  3. Attention per head pair, scoresT [k-part, q-free]: DoubleRow scores
     (zero slab), exp on ACT into fp8 pT (kc-pair tiles), causal diag masked
     by DVE multiply, ctx accumulated DoubleRow over kc pairs.
     1/den via DVE reciprocal (row 64), broadcast across partitions by a
     1-row matmul against a ones-at-row-64 column, DVE normalize -> bf16.
  4. Two bf16 AllToAlls (one per head pair, 1MB each) across all 8 cores,
     shards duplicated across batch halves. Receiver picks its batch half
     with a DVE mask-select driven by the bsel input (no gather).
  5. out = ctxT.T @ Wo + bo in bf16, pair-0 partials during the second
     AllToAll, DMA to y [512, 1024] fp32.
"""

import contextlib
import ctypes
import sys
import types

sys.path.insert(0, "/opt/trn_rl_repo")

import numpy as np
import ml_dtypes

import concourse.bass as bass
import concourse.mybir as mybir
import concourse.tile as tile
from concourse import bacc
from concourse import bass_utils

P = 128
B, S, D = 2, 2048, 1024
NH_LOC = 4          # heads per core
HD = 64             # head dim
G = NH_LOC * HD     # local head dims = 256
MC = G // P         # m-chunks of local dims = 2
DC = D // P         # d-chunks = 8
TB = 512            # token block (output tokens per core, q-tile width)
NQT = S // TB       # q-tiles = 4
NTC = S // P        # token chunks = 16
NC = 8
VP = 80             # per-head v block stride (64 data + 1 ones + 15 pad)

F32 = mybir.dt.float32
F32R = mybir.dt.float32r
BF16 = mybir.dt.bfloat16
F8 = mybir.dt.float8e4
Exp = mybir.ActivationFunctionType.Exp
Ident = mybir.ActivationFunctionType.Identity
MULT = mybir.AluOpType.mult
ADD = mybir.AluOpType.add
DR = mybir.MatmulPerfMode.DoubleRow

# per-stage dtype switches (bf16 fallback for numerics bisection)
FP8_QKV = False
FP8_SC = False
FP8_CTX = False
WSC = 64.0 if FP8_QKV else 1.0  # host weight prescale

DX = F8 if FP8_QKV else BF16    # xt, wq/wk/wv
DQK = F8 if FP8_SC else BF16    # stored q / k
DP = F8 if FP8_CTX else BF16    # pT probs and v

NP_BF16 = np.dtype(ml_dtypes.bfloat16)
NP_F8 = np.dtype(ml_dtypes.float8_e4m3)


def _install_ntff_hook():
    """Make trace=True work under axon: inject antenv.axon_hooks backed by
    ctypes calls into libaxon_pjrt.so (mirrors trn_agent_boot logic)."""
    if "antenv.axon_hooks" in sys.modules:
        return
    holder = {}
    mod = types.ModuleType("antenv.axon_hooks")
    mod.set_axon_ntff_profile_hook = lambda h: holder.update(h=h)
    mod.get_axon_ntff_profile_hook = lambda: holder.get("h")
    sys.modules["antenv.axon_hooks"] = mod
    try:
        lib = ctypes.CDLL("/opt/axon/libaxon_pjrt.so")
        if not hasattr(lib, "axon_start_nrt_profile"):
            return
    except OSError:
        return
    lib.axon_start_nrt_profile.argtypes = [
        ctypes.POINTER(ctypes.c_int64),
        ctypes.c_size_t,
    ]
    lib.axon_start_nrt_profile.restype = ctypes.c_int64
    lib.axon_stop_nrt_profile.argtypes = [ctypes.c_char_p]
    lib.axon_stop_nrt_profile.restype = ctypes.c_int64

    @contextlib.contextmanager
    def _hook(output_dir, device_ids):
        import jax

        jax.devices()
        if device_ids:
            ids = (ctypes.c_int64 * len(device_ids))(*device_ids)
            rc = lib.axon_start_nrt_profile(ids, len(device_ids))
        else:
            rc = lib.axon_start_nrt_profile(None, 0)
        if rc != 0:
            raise RuntimeError(f"axon_start_nrt_profile rc={rc}")
        try:
            yield
        finally:
            n = lib.axon_stop_nrt_profile(str(output_dir).encode())
            print(f"profile: {n} ntff file(s) written to {output_dir}")

    holder["h"] = _hook


def build(coll=True):
    nc = bacc.Bacc("TRN2", target_bir_lowering=False, debug=False, num_devices=NC)

    xt_d = nc.dram_tensor("xt", [D, S], DX, kind="ExternalInput").ap()
    wq_d = nc.dram_tensor("wq", [D, G], DX, kind="ExternalInput").ap()
    wk_d = nc.dram_tensor("wk", [D, G], DX, kind="ExternalInput").ap()
    wv_d = nc.dram_tensor("wv", [D, G], DX, kind="ExternalInput").ap()
    bq_d = nc.dram_tensor("bq", [P, MC], F32, kind="ExternalInput").ap()
    bk_d = nc.dram_tensor("bk", [P, MC], F32, kind="ExternalInput").ap()
    bv_d = nc.dram_tensor("bv", [1, G], F32, kind="ExternalInput").ap()
    wo_d = nc.dram_tensor("wo", [D, D], BF16, kind="ExternalInput").ap()
    bo_d = nc.dram_tensor("bo", [1, D], F32, kind="ExternalInput").ap()
    bsel_d = nc.dram_tensor("bsel", [P, 2], F32, kind="ExternalInput").ap()
    y_d = nc.dram_tensor("y", [TB, D], F32, kind="ExternalOutput").ap()

    with tile.TileContext(nc) as tc:
        with (
            tc.tile_pool(name="const", bufs=1) as const,
            tc.tile_pool(name="dram", bufs=1, space="DRAM") as dram,
            tc.tile_pool(name="ps_mm", bufs=2, space="PSUM") as ps_mm,
            tc.tile_pool(name="ps_sm", bufs=1, space="PSUM") as ps_sm,
            tc.tile_pool(name="ps_ctx", bufs=3, space="PSUM") as ps_ctx,
            tc.tile_pool(name="persist", bufs=1) as persist,
        ):
            # ---------------- constants ----------------
            ones_f = const.tile([P, 1], F32, tag="ones_f")
            nc.vector.memset(ones_f[:], 1.0)
            # trimask[k, u] = 1 if k <= u else 0 (keep where u - k >= 0)
            tri_f = const.tile([P, P], F32, tag="tri_f")
            nc.gpsimd.memset(tri_f[:], 1.0)
            nc.gpsimd.affine_select(
                out=tri_f[:],
                in_=tri_f[:],
                compare_op=mybir.AluOpType.is_ge,
                fill=0.0,
                base=0,
                pattern=[[1, P]],
                channel_multiplier=-1,
            )
            # materialized for both heads so the gpsimd mask multiply uses a
            # plain strided AP (no broadcast)
            tri_p = const.tile([P, 2, P], DP, tag="tri_p")
            nc.vector.tensor_copy(
                tri_p[:], tri_f[:, None, :].to_broadcast((P, 2, P))
            )
            # ones at row 64 only: broadcast-den matmul weights
            zrow_f = const.tile([P, HD], F32, tag="zrow_f")
            nc.vector.memset(zrow_f[:], 0.0)
            nc.vector.memset(zrow_f[64:65, :], 1.0)
            onescol_r = const.tile([P, HD], F32R, tag="onescol_r")
            nc.vector.tensor_copy(onescol_r[:], zrow_f[:])
            zeros_f = const.tile([P, 512], F32, tag="zeros_f")
            nc.vector.memset(zeros_f[:], 0.0)

            bq_sb = const.tile([P, MC], F32, tag="bq")
            bk_sb = const.tile([P, MC], F32, tag="bk")
            nc.sync.dma_start(bq_sb[:], bq_d)
            nc.sync.dma_start(bk_sb[:], bk_d)
            bv_row = const.tile([1, G], F32, tag="bv_row")
            nc.sync.dma_start(bv_row[:], bv_d)
            bv_bc = const.tile([P, G], F32, tag="bv_bc")
            nc.gpsimd.partition_broadcast(bv_bc[:], bv_row[:])
            bo_row = const.tile([1, D], F32, tag="bo_row")
            bo_bc = const.tile([P, D], F32, tag="bo_bc")
            bsel_sb = const.tile([P, 2], F32, tag="bsel")
            nc.sync.dma_start(bsel_sb[:], bsel_d)

            # persistent activations: q/k [p, pair, kslab, tok] with kslab 1
            # zeroed (DoubleRow zero-pad), v [p, tc, head, 80] fp8
            KSL = 2 if FP8_SC else 1
            qT = persist.tile([P, MC, KSL, S], DQK, tag="qT")
            kT = persist.tile([P, MC, KSL, S], DQK, tag="kT")
            v_sb = persist.tile([P, NTC, NH_LOC, VP], DP, tag="v")
            wo_sb = persist.tile([P, DC, D], BF16, tag="wo")

            if FP8_SC:
                nc.gpsimd.memset(qT[:, :, 1, :], 0.0)
                nc.gpsimd.memset(kT[:, :, 1, :], 0.0)
            # ones column of v (denominator trick): col 64 of each head block
            nc.gpsimd.memset(v_sb[:, :, :, HD : HD + 1], 1.0)

            a2a_in = [
                dram.tile([NC * P, TB], BF16, name=f"a2ain{p}", tag=f"a2ain{p}")
                for p in range(2)
            ]
            a2a_out = [
                dram.tile([NC * P, TB], BF16, name=f"a2aout{p}", tag=f"a2aout{p}")
                for p in range(2)
            ]

            with (
                tc.tile_pool(name="xw", bufs=1) as xw,
            ):
                wq_sb = xw.tile([P, DC, G], DX, tag="wq")
                wk_sb = xw.tile([P, DC, G], DX, tag="wk")
                wv_sb = xw.tile([P, DC, G], DX, tag="wv")
                nc.sync.dma_start(wq_sb[:], wq_d.rearrange("(dc p) m -> p dc m", p=P))

                xTt = [
                    xw.tile([P, DC, TB], DX, tag=f"xT{g}", name=f"xT{g}")
                    for g in range(NQT)
                ]
                xTg = [t[:] for t in xTt]
                xt_r = xt_d.rearrange("(dc p) t -> p dc t", p=P)

                for g in range(NQT):
                    nc.sync.dma_start(
                        xTg[g], xt_r[:, :, g * TB : (g + 1) * TB]
                    )
                    if g == 0:
                        nc.sync.dma_start(
                            wk_sb[:], wk_d.rearrange("(dc p) m -> p dc m", p=P)
                        )
                        nc.sync.dma_start(
                            wv_sb[:], wv_d.rearrange("(dc p) m -> p dc m", p=P)
                        )

                def emit_qkv(g):
                    for w_sb, b_sb, out_t in ((wq_sb, bq_sb, qT), (wk_sb, bk_sb, kT)):
                        for mc_i in range(MC):
                            pj = ps_sm.tile([P, 512], F32, tag="sm")
                            if FP8_QKV:
                                for i, dc in enumerate(range(0, DC, 2)):
                                    nc.tensor.matmul(
                                        pj[:],
                                        w_sb[:, dc : dc + 2, mc_i * P : (mc_i + 1) * P],
                                        xTg[g][:, dc : dc + 2, :],
                                        start=(i == 0),
                                        stop=(dc == DC - 2),
                                        perf_mode=DR,
                                    )
                            else:
                                for dc in range(DC):
                                    nc.tensor.matmul(
                                        pj[:],
                                        w_sb[:, dc, mc_i * P : (mc_i + 1) * P],
                                        xTg[g][:, dc, :],
                                        start=(dc == 0),
                                        stop=(dc == DC - 1),
                                    )
                            nc.scalar.activation(
                                out_t[:, mc_i, 0, g * TB : (g + 1) * TB],
                                pj[:],
                                Ident,
                                bias=b_sb[:, mc_i : mc_i + 1],
                                scale=1.0 / WSC,
                            )
                    for ti in range(4):
                        tc_i = 4 * g + ti
                        pv = ps_sm.tile([P, 512], F32, tag="sm")
                        if FP8_QKV:
                            for i, dc in enumerate(range(0, DC, 2)):
                                nc.tensor.matmul(
                                    pv[:, 0:G],
                                    xTg[g][:, dc : dc + 2, ti * P : (ti + 1) * P],
                                    wv_sb[:, dc : dc + 2, :],
                                    start=(i == 0),
                                    stop=(dc == DC - 2),
                                    perf_mode=DR,
                                )
                        else:
                            for dc in range(DC):
                                nc.tensor.matmul(
                                    pv[:, 0:G],
                                    xTg[g][:, dc, ti * P : (ti + 1) * P],
                                    wv_sb[:, dc, :],
                                    start=(dc == 0),
                                    stop=(dc == DC - 1),
                                )
                        # v = pv/WSC + bv, fp8 store into padded head blocks
                        nc.vector.scalar_tensor_tensor(
                            v_sb[:, tc_i, :, 0:HD],
                            pv[:, 0:G].rearrange("p (h c) -> p h c", c=HD),
                            1.0 / WSC,
                            bv_bc[:].rearrange("p (h c) -> p h c", c=HD),
                            MULT,
                            ADD,
                        )

                # ------ attention, emission-interleaved with QKV blocks ------
                nc.sync.dma_start(
                    wo_sb[:], wo_d.rearrange("(dc p) n -> p dc n", p=P)
                )
                nc.sync.dma_start(bo_row[:], bo_d)
                nc.gpsimd.partition_broadcast(bo_bc[:], bo_row[:])
                work = tc.alloc_tile_pool(name="att", bufs=1)
                pTp = tc.alloc_tile_pool(name="pTp", bufs=8)
                smallp = tc.alloc_tile_pool(name="smallp", bufs=4)
                ctxn = [
                    [
                        work.tile(
                            [HD, TB], BF16, tag=f"ctxn{h}_{q}", name=f"ctxn{h}_{q}"
                        )
                        for q in range(NQT)
                    ]
                    for h in range(NH_LOC)
                ]
                # reciprocal of denominator lives at row 64; other rows stay 0
                # so the broadcast matmul (ones at row 64) reads no garbage
                rdenX = [
                    work.tile([P, 512], F32, tag=f"rdenX{i}", name=f"rdenX{i}")
                    for i in range(2)
                ]
                for i in range(2):
                    nc.vector.tensor_copy(rdenX[i][:], zeros_f[:])
                c_ps_of = {}

                def emit_scores(pair, ch):
                    qt, k0, k1 = ch
                    p_tiles = {}
                    for kc in range(k0, k1):
                        j = kc - 4 * qt
                        coff = max(0, j) * P
                        if kc % 2 == 0:
                            pT = pTp.tile([P, 2, 2, TB], DP, tag="pT")
                            p_tiles[kc] = pT
                        else:
                            pT = p_tiles[kc - 1]
                        s_ps = ps_mm.tile([P, 2, 512], F32, tag="mm")
                        for h01 in range(2):
                            pb = h01 * HD
                            if FP8_SC:
                                nc.tensor.matmul(
                                    s_ps[:, h01, coff:512],
                                    kT[pb : pb + HD, pair, :, kc * P : (kc + 1) * P],
                                    qT[
                                        pb : pb + HD,
                                        pair,
                                        :,
                                        qt * TB + coff : (qt + 1) * TB,
                                    ],
                                    start=True,
                                    stop=True,
                                    perf_mode=DR,
                                )
                            else:
                                nc.tensor.matmul(
                                    s_ps[:, h01, coff:512],
                                    kT[pb : pb + HD, pair, 0, kc * P : (kc + 1) * P],
                                    qT[
                                        pb : pb + HD,
                                        pair,
                                        0,
                                        qt * TB + coff : (qt + 1) * TB,
                                    ],
                                    start=True,
                                    stop=True,
                                )
                        nc.scalar.activation(
                            pT[:, kc % 2, :, coff:512],
                            s_ps[:, :, coff:512],
                            Exp,
                            scale=0.125,
                        )
                        if j >= 0:
                            nc.gpsimd.tensor_tensor(
                                pT[:, kc % 2, :, coff : coff + P],
                                pT[:, kc % 2, :, coff : coff + P],
                                tri_p[:],
                                MULT,
                            )
                    return p_tiles

                def emit_ctx(pair, ch, p_tiles):
                    qt, k0, k1 = ch
                    nkc = 4 * qt + 4
                    if k0 == 0:
                        c_ps_of[pair, qt] = [
                            ps_ctx.tile([P, 512], F32, tag="ctx", name=f"cps{h01}")
                            for h01 in range(2)
                        ]
                    c_ps = c_ps_of[pair, qt]
                    for kc in range(k0, k1):
                        j = kc - 4 * qt
                        pT = p_tiles[kc - kc % 2]
                        coff = max(0, j) * P
                        for h01 in range(2):
                            h = 2 * pair + h01
                            nc.tensor.matmul(
                                c_ps[h01][0 : HD + 1, coff:512],
                                v_sb[:, kc, h, 0 : HD + 1],
                                pT[:, kc % 2, h01, coff:512],
                                start=(kc == 0),
                                stop=(kc == nkc - 1),
                            )
                def emit_norm(pair, qt):
                    # normalize by 1/den (den = ones-row sums at psum row 64),
                    # broadcast across partitions via a row-64 matmul. The two
                    # heads' reciprocals run on different engines (DVE and
                    # ACT Ln->Exp) so they don't serialize.
                    c_ps = c_ps_of[pair, qt]
                    for h01 in range(2):
                        rX = rdenX[h01]
                        if h01 == 0:
                            with nc.allow_low_precision(reason="den recip"):
                                nc.vector.reciprocal(
                                    rX[64:65, :], c_ps[h01][64:65, :]
                                )
                        else:
                            lnd = smallp.tile([P, 512], F32, tag="lnd")
                            nc.scalar.activation(
                                lnd[64:65, :],
                                c_ps[h01][64:65, :],
                                mybir.ActivationFunctionType.Ln,
                            )
                            nc.scalar.activation(
                                rX[64:65, :], lnd[64:65, :], Exp, scale=-1.0
                            )
                    for h01 in range(2):
                        h = 2 * pair + h01
                        b_ps = ps_sm.tile([P, 512], F32, tag="sm", name="bps")
                        nc.tensor.matmul(
                            b_ps[0:HD, :],
                            zrow_f[:, 0:HD],
                            rdenX[h01][:],
                            start=True,
                            stop=True,
                        )
                        bb = smallp.tile([HD, 512], F32, tag="bb")
                        nc.scalar.copy(bb[:], b_ps[0:HD, :])
                        nc.vector.tensor_tensor(
                            ctxn[h][qt][:, :],
                            c_ps[h01][0:HD, :],
                            bb[:],
                            MULT,
                        )
                    # A2A sends for this (pair, qt): destination block qt,
                    # duplicated across batch halves
                    for sh in (qt, qt + 4):
                        for h01 in range(2):
                            h = 2 * pair + h01
                            nc.sync.dma_start(
                                a2a_in[pair][
                                    sh * P + h01 * HD : sh * P + (h01 + 1) * HD,
                                    :,
                                ],
                                ctxn[h][qt][:, :],
                            )

                def qt_chunks(qt):
                    return [
                        (qt, kcb, min(kcb + 8, 4 * qt + 4))
                        for kcb in range(0, 4 * qt + 4, 8)
                    ]

                # software pipeline: emit scores(i+1) before ctx(i) so the PE
                # queue never blocks the ACT exp stream behind ctx matmuls;
                # pair-0 attention interleaves with the QKV q-block loop
                pend = []
                norms = []

                def push(pair, ch):
                    tiles = emit_scores(pair, ch)
                    # norms lag one more chunk than ctx so the reciprocal
                    # chain never blocks the PE queue at the bcast matmul
                    while norms:
                        emit_norm(*norms.pop(0))
                    if pend:
                        p2, c2, t2 = pend.pop()
                        emit_ctx(p2, c2, t2)
                        if c2[2] == 4 * c2[0] + 4:
                            norms.append((p2, c2[0]))
                    pend.append((pair, ch, tiles))

                def flush():
                    p2, c2, t2 = pend.pop()
                    emit_ctx(p2, c2, t2)
                    if c2[2] == 4 * c2[0] + 4:
                        norms.append((p2, c2[0]))
                    while norms:
                        emit_norm(*norms.pop(0))

                for g in range(NQT):
                    emit_qkv(g)
                    for ch in qt_chunks(g):
                        push(0, ch)
                flush()
                if coll:
                    nc.gpsimd.collective_compute(
                        "AllToAll",
                        mybir.AluOpType.bypass,
                        ins=[a2a_in[0].opt()],
                        outs=[a2a_out[0].opt()],
                        replica_groups=[list(range(NC))],
                    )
                for qt in range(NQT):
                    for ch in qt_chunks(qt):
                        push(1, ch)
                flush()
                if coll:
                    nc.gpsimd.collective_compute(
                        "AllToAll",
                        mybir.AluOpType.bypass,
                        ins=[a2a_in[1].opt()],
                        outs=[a2a_out[1].opt()],
                        replica_groups=[list(range(NC))],
                    )

                smallp.release()
                pTp.release()
                work.release()

            # ---------- receive + output projection ----------
            outp = tc.alloc_tile_pool(name="outp", bufs=1)
            gsrc = a2a_out if coll else a2a_in
            # cx[p, pr, j, t]: sender j's pair-pr dims for my token block
            cx = outp.tile([P, 2, NC, TB], BF16, tag="cx")
            tmp = outp.tile([P, NQT, TB], BF16, tag="seltmp")
            # ctxf[p, pr, g, t]: global dim chunk dc = 2*g + pr
            ctxf = outp.tile([P, 2, NQT, TB], BF16, tag="ctxf")

            with tc.tile_pool(name="out_pool", bufs=3) as out_pool:
                o_parts = [
                    outp.tile([P, 512], F32, tag=f"opart{u}", name=f"opart{u}")
                    for u in range(8)
                ]
                for pr in range(2):
                    nc.sync.dma_start(
                        cx[:, pr],
                        gsrc[pr][:].rearrange("(j q) t -> q j t", q=P),
                    )
                # batch select: ctxf = lo*bsel[0] + hi*bsel[1]
                nc.vector.tensor_scalar(
                    tmp[:], cx[:, 0, 0:4], bsel_sb[:, 0:1], None, MULT
                )
                nc.vector.scalar_tensor_tensor(
                    ctxf[:, 0], cx[:, 0, 4:8], bsel_sb[:, 1:2], tmp[:], MULT, ADD
                )
                # pair-0 (even) chunks first: overlap with A2A of pair 1
                for u in range(8):
                    tc_i, nt = u // 2, u % 2
                    po = ps_mm.tile([P, 2, 512], F32, tag="mm")
                    for i, g in enumerate(range(NQT)):
                        nc.tensor.matmul(
                            po[:, 0, :],
                            ctxf[:, 0, g, tc_i * P : (tc_i + 1) * P],
                            wo_sb[:, 2 * g, nt * 512 : (nt + 1) * 512],
                            start=(i == 0),
                            stop=(i == NQT - 1),
                        )
                    nc.vector.tensor_tensor(
                        o_parts[u][:],
                        po[:, 0, :],
                        bo_bc[:, nt * 512 : (nt + 1) * 512],
                        ADD,
                    )
                nc.vector.tensor_scalar(
                    tmp[:], cx[:, 1, 0:4], bsel_sb[:, 0:1], None, MULT
                )
                nc.vector.scalar_tensor_tensor(
                    ctxf[:, 1], cx[:, 1, 4:8], bsel_sb[:, 1:2], tmp[:], MULT, ADD
                )
                for u in range(8):
                    tc_i, nt = u // 2, u % 2
                    po = ps_mm.tile([P, 2, 512], F32, tag="mm")
                    for i, g in enumerate(range(NQT)):
                        nc.tensor.matmul(
                            po[:, 0, :],
                            ctxf[:, 1, g, tc_i * P : (tc_i + 1) * P],
                            wo_sb[:, 2 * g + 1, nt * 512 : (nt + 1) * 512],
                            start=(i == 0),
                            stop=(i == NQT - 1),
                        )
                    o_sb = out_pool.tile([P, 512], F32, tag="osb")
                    nc.vector.tensor_tensor(
                        o_sb[:], po[:, 0, :], o_parts[u][:], ADD
                    )
                    nc.sync.dma_start(
                        y_d[
                            tc_i * P : (tc_i + 1) * P,
                            nt * 512 : (nt + 1) * 512,
                        ],
                        o_sb[:],
                    )

            outp.release()

    nc.compile()
    return nc


_NC_CACHE = {}


def _get_nc():
    if "nc" not in _NC_CACHE:
        _NC_CACHE["nc"] = build()
    return _NC_CACHE["nc"]


def _make_in_maps(x, Wq, bq, Wk, bk, Wv, bv, Wo, bo):
    x = np.asarray(x, np.float32)
    Wq, Wk, Wv, Wo = (np.asarray(a, np.float32) for a in (Wq, Wk, Wv, Wo))
    bq, bk, bv, bo = (np.asarray(a, np.float32) for a in (bq, bk, bv, bo))
    np_dx = NP_F8 if FP8_QKV else NP_BF16
    wo_b = np.ascontiguousarray(Wo).astype(NP_BF16)
    in_maps = []
    for c in range(NC):
        b, g = c // 4, c % 4
        sl = slice(g * G, (g + 1) * G)
        bsel = np.tile(
            np.array([1.0 - b, float(b)], np.float32).reshape(1, 2), (P, 1)
        )
        in_maps.append(
            {
                "xt": np.ascontiguousarray(x[b].T).astype(np_dx),
                "wq": np.ascontiguousarray(Wq[:, sl] * WSC).astype(np_dx),
                "wk": np.ascontiguousarray(Wk[:, sl] * WSC).astype(np_dx),
                "wv": np.ascontiguousarray(Wv[:, sl] * WSC).astype(np_dx),
                "bq": np.ascontiguousarray(bq[sl].reshape(MC, P).T),
                "bk": np.ascontiguousarray(bk[sl].reshape(MC, P).T),
                "bv": np.ascontiguousarray(bv[sl].reshape(1, G)),
                "wo": wo_b,
                "bo": np.ascontiguousarray(bo.reshape(1, D)),
                "bsel": np.ascontiguousarray(bsel),
            }
        )
    return in_maps


def run(inputs, trace=False, tmpdir=None):
    """Run on 8 cores; returns (output [2,2048,1024], BassKernelResults)."""
    if trace:
        _install_ntff_hook()
    nc = _get_nc()
    in_maps = _make_in_maps(**inputs)
    res = bass_utils.run_bass_kernel_spmd(
        nc, in_maps, core_ids=list(range(NC)), trace=trace, tmpdir=tmpdir
    )
    out = np.empty((B, S, D), np.float32)
    for c in range(NC):
        b, g = c // 4, c % 4
        out[b, g * TB : (g + 1) * TB, :] = res.results[c]["y"]
    return out, res


def kernel(**inputs) -> np.ndarray:
    out, _ = run(inputs, trace=False)
    return out


# revision 47
# speedup vs baseline: 1.0474x; 1.0474x over previous
"""GPT self-attention layer (B=2, S=2048, D=1024, H=16, hd=64) on 8 TRN2 cores.

Sharding: data-parallel over batch (2) x tensor-parallel over heads (4 groups
of 4 heads). Core c handles batch b=c//4, head group g=c%4.

v2: fp8e4m3 DoubleRow matmuls for QKV / scores / ctx (4x / 2x / 4x PE rate),
bf16 out-projection and bf16 AllToAll payloads, softmax reciprocal on DVE,
receiver-side batch select on DVE (replaces gpsimd dma_gather).

Per-core pipeline:
  1. xt = x[b].T arrives fp8 (host cast); wq/wk/wv fp8 prescaled by 64 (fp8
     subnormal dodge), descaled for free in the ACT bias op / DVE V-store.
  2. QT/KT = W.T @ x.T via DoubleRow (dc pairs), stored fp8 with a zeroed
     second k-subtile slab so the scores matmul can run DoubleRow with
     contraction 64. V = x @ Wv stored fp8 with interleaved ones column
     (softmax denominator rides along in ctx psum row 64).
  3. Attention per head pair, scoresT [k-part, q-free]: DoubleRow scores
     (zero slab), exp on ACT into fp8 pT (kc-pair tiles), causal diag masked
     by DVE multiply, ctx accumulated DoubleRow over kc pairs.
     1/den via DVE reciprocal (row 64), broadcast across partitions by a
     1-row matmul against a ones-at-row-64 column, DVE normalize -> bf16.
  4. Two bf16 AllToAlls (one per head pair, 1MB each) across all 8 cores,
     shards duplicated across batch halves. Receiver picks its batch half
     with a DVE mask-select driven by the bsel input (no gather).
  5. out = ctxT.T @ Wo + bo in bf16, pair-0 partials during the second
     AllToAll, DMA to y [512, 1024] fp32.
"""

import contextlib
import ctypes
import sys
import types

sys.path.insert(0, "/opt/trn_rl_repo")

import numpy as np
import ml_dtypes

import concourse.bass as bass
import concourse.mybir as mybir
import concourse.tile as tile
from concourse import bacc
from concourse import bass_utils

P = 128
B, S, D = 2, 2048, 1024
NH_LOC = 4          # heads per core
HD = 64             # head dim
G = NH_LOC * HD     # local head dims = 256
MC = G // P         # m-chunks of local dims = 2
DC = D // P         # d-chunks = 8
TB = 512            # token block (output tokens per core, q-tile width)
NQT = S // TB       # q-tiles = 4
NTC = S // P        # token chunks = 16
NC = 8
VP = 80             # per-head v block stride (64 data + 1 ones + 15 pad)

F32 = mybir.dt.float32
F32R = mybir.dt.float32r
BF16 = mybir.dt.bfloat16
F8 = mybir.dt.float8e4
Exp = mybir.ActivationFunctionType.Exp
Ident = mybir.ActivationFunctionType.Identity
MULT = mybir.AluOpType.mult
ADD = mybir.AluOpType.add
DR = mybir.MatmulPerfMode.DoubleRow

# per-stage dtype switches (bf16 fallback for numerics bisection)
FP8_QKV = False
FP8_SC = False
FP8_CTX = False
WSC = 64.0 if FP8_QKV else 1.0  # host weight prescale

DX = F8 if FP8_QKV else BF16    # xt, wq/wk/wv
DQK = F8 if FP8_SC else BF16    # stored q / k
DP = F8 if FP8_CTX else BF16    # pT probs and v

NP_BF16 = np.dtype(ml_dtypes.bfloat16)
NP_F8 = np.dtype(ml_dtypes.float8_e4m3)


def _install_ntff_hook():
    """Make trace=True work under axon: inject antenv.axon_hooks backed by
    ctypes calls into libaxon_pjrt.so (mirrors trn_agent_boot logic)."""
    if "antenv.axon_hooks" in sys.modules:
        return
    holder = {}
    mod = types.ModuleType("antenv.axon_hooks")
    mod.set_axon_ntff_profile_hook = lambda h: holder.update(h=h)
    mod.get_axon_ntff_profile_hook = lambda: holder.get("h")
    sys.modules["antenv.axon_hooks"] = mod
    try:
        lib = ctypes.CDLL("/opt/axon/libaxon_pjrt.so")
        if not hasattr(lib, "axon_start_nrt_profile"):
            return
    except OSError:
        return
    lib.axon_start_nrt_profile.argtypes = [
        ctypes.POINTER(ctypes.c_int64),
        ctypes.c_size_t,
    ]
    lib.axon_start_nrt_profile.restype = ctypes.c_int64
    lib.axon_stop_nrt_profile.argtypes = [ctypes.c_char_p]
    lib.axon_stop_nrt_profile.restype = ctypes.c_int64

    @contextlib.contextmanager
    def _hook(output_dir, device_ids):
        import jax

        jax.devices()
        if device_ids:
            ids = (ctypes.c_int64 * len(device_ids))(*device_ids)
            rc = lib.axon_start_nrt_profile(ids, len(device_ids))
        else:
            rc = lib.axon_start_nrt_profile(None, 0)
        if rc != 0:
            raise RuntimeError(f"axon_start_nrt_profile rc={rc}")
        try:
            yield
        finally:
            n = lib.axon_stop_nrt_profile(str(output_dir).encode())
            print(f"profile: {n} ntff file(s) written to {output_dir}")

    holder["h"] = _hook


def build(coll=True):
    nc = bacc.Bacc("TRN2", target_bir_lowering=False, debug=False, num_devices=NC)

    xt_d = nc.dram_tensor("xt", [D, S], DX, kind="ExternalInput").ap()
    wq_d = nc.dram_tensor("wq", [D, G], DX, kind="ExternalInput").ap()
    wk_d = nc.dram_tensor("wk", [D, G], DX, kind="ExternalInput").ap()
    wv_d = nc.dram_tensor("wv", [D, G], DX, kind="ExternalInput").ap()
    bq_d = nc.dram_tensor("bq", [P, MC], F32, kind="ExternalInput").ap()
    bk_d = nc.dram_tensor("bk", [P, MC], F32, kind="ExternalInput").ap()
    bv_d = nc.dram_tensor("bv", [1, G], F32, kind="ExternalInput").ap()
    wo_d = nc.dram_tensor("wo", [D, D], BF16, kind="ExternalInput").ap()
    bo_d = nc.dram_tensor("bo", [1, D], F32, kind="ExternalInput").ap()
    bsel_d = nc.dram_tensor("bsel", [P, 2], F32, kind="ExternalInput").ap()
    y_d = nc.dram_tensor("y", [TB, D], F32, kind="ExternalOutput").ap()

    with tile.TileContext(nc) as tc:
        with (
            tc.tile_pool(name="const", bufs=1) as const,
            tc.tile_pool(name="dram", bufs=1, space="DRAM") as dram,
            tc.tile_pool(name="ps_mm", bufs=2, space="PSUM") as ps_mm,
            tc.tile_pool(name="ps_sm", bufs=1, space="PSUM") as ps_sm,
            tc.tile_pool(name="ps_ctx", bufs=3, space="PSUM") as ps_ctx,
            tc.tile_pool(name="persist", bufs=1) as persist,
        ):
            # ---------------- constants ----------------
            ones_f = const.tile([P, 1], F32, tag="ones_f")
            nc.vector.memset(ones_f[:], 1.0)
            # trimask[k, u] = 1 if k <= u else 0 (keep where u - k >= 0)
            tri_f = const.tile([P, P], F32, tag="tri_f")
            nc.gpsimd.memset(tri_f[:], 1.0)
            nc.gpsimd.affine_select(
                out=tri_f[:],
                in_=tri_f[:],
                compare_op=mybir.AluOpType.is_ge,
                fill=0.0,
                base=0,
                pattern=[[1, P]],
                channel_multiplier=-1,
            )
            # materialized for both heads so the gpsimd mask multiply uses a
            # plain strided AP (no broadcast)
            tri_p = const.tile([P, 2, P], DP, tag="tri_p")
            nc.vector.tensor_copy(
                tri_p[:], tri_f[:, None, :].to_broadcast((P, 2, P))
            )
            # ones at row 64 only: broadcast-den matmul weights
            zrow_f = const.tile([P, HD], F32, tag="zrow_f")
            nc.vector.memset(zrow_f[:], 0.0)
            nc.vector.memset(zrow_f[64:65, :], 1.0)
            onescol_r = const.tile([P, HD], F32R, tag="onescol_r")
            nc.vector.tensor_copy(onescol_r[:], zrow_f[:])
            zeros_f = const.tile([P, 512], F32, tag="zeros_f")
            nc.vector.memset(zeros_f[:], 0.0)

            bq_sb = const.tile([P, MC], F32, tag="bq")
            bk_sb = const.tile([P, MC], F32, tag="bk")
            nc.sync.dma_start(bq_sb[:], bq_d)
            nc.sync.dma_start(bk_sb[:], bk_d)
            bv_row = const.tile([1, G], F32, tag="bv_row")
            nc.sync.dma_start(bv_row[:], bv_d)
            bv_bc = const.tile([P, G], F32, tag="bv_bc")
            nc.gpsimd.partition_broadcast(bv_bc[:], bv_row[:])
            bo_row = const.tile([1, D], F32, tag="bo_row")
            bo_bc = const.tile([P, D], F32, tag="bo_bc")
            bsel_sb = const.tile([P, 2], F32, tag="bsel")
            nc.sync.dma_start(bsel_sb[:], bsel_d)

            # persistent activations: q/k [p, pair, kslab, tok] with kslab 1
            # zeroed (DoubleRow zero-pad), v [p, tc, head, 80] fp8
            KSL = 2 if FP8_SC else 1
            qT = persist.tile([P, MC, KSL, S], DQK, tag="qT")
            kT = persist.tile([P, MC, KSL, S], DQK, tag="kT")
            v_sb = persist.tile([P, NTC, NH_LOC, VP], DP, tag="v")
            wo_sb = persist.tile([P, DC, D], BF16, tag="wo")

            if FP8_SC:
                nc.gpsimd.memset(qT[:, :, 1, :], 0.0)
                nc.gpsimd.memset(kT[:, :, 1, :], 0.0)
            # ones column of v (denominator trick): col 64 of each head block
            nc.gpsimd.memset(v_sb[:, :, :, HD : HD + 1], 1.0)

            a2a_in = [
                dram.tile([NC * P, TB], BF16, name=f"a2ain{p}", tag=f"a2ain{p}")
                for p in range(2)
            ]
            a2a_out = [
                dram.tile([NC * P, TB], BF16, name=f"a2aout{p}", tag=f"a2aout{p}")
                for p in range(2)
            ]

            with (
                tc.tile_pool(name="xw", bufs=1) as xw,
            ):
                wq_sb = xw.tile([P, DC, G], DX, tag="wq")
                wk_sb = xw.tile([P, DC, G], DX, tag="wk")
                wv_sb = xw.tile([P, DC, G], DX, tag="wv")
                nc.sync.dma_start(wq_sb[:], wq_d.rearrange("(dc p) m -> p dc m", p=P))

                xTt = [
                    xw.tile([P, DC, TB], DX, tag=f"xT{g}", name=f"xT{g}")
                    for g in range(NQT)
                ]
                xTg = [t[:] for t in xTt]
                xt_r = xt_d.rearrange("(dc p) t -> p dc t", p=P)

                for g in range(NQT):
                    nc.sync.dma_start(
                        xTg[g], xt_r[:, :, g * TB : (g + 1) * TB]
                    )
                    if g == 0:
                        nc.sync.dma_start(
                            wk_sb[:], wk_d.rearrange("(dc p) m -> p dc m", p=P)
                        )
                        nc.sync.dma_start(
                            wv_sb[:], wv_d.rearrange("(dc p) m -> p dc m", p=P)
                        )

                def emit_qkv(g):
                    for w_sb, b_sb, out_t in ((wq_sb, bq_sb, qT), (wk_sb, bk_sb, kT)):
                        for mc_i in range(MC):
                            pj = ps_sm.tile([P, 512], F32, tag="sm")
                            if FP8_QKV:
                                for i, dc in enumerate(range(0, DC, 2)):
                                    nc.tensor.matmul(
                                        pj[:],
                                        w_sb[:, dc : dc + 2, mc_i * P : (mc_i + 1) * P],
                                        xTg[g][:, dc : dc + 2, :],
                                        start=(i == 0),
                                        stop=(dc == DC - 2),
                                        perf_mode=DR,
                                    )
                            else:
                                for dc in range(DC):
                                    nc.tensor.matmul(
                                        pj[:],
                                        w_sb[:, dc, mc_i * P : (mc_i + 1) * P],
                                        xTg[g][:, dc, :],
                                        start=(dc == 0),
                                        stop=(dc == DC - 1),
                                    )
                            nc.scalar.activation(
                                out_t[:, mc_i, 0, g * TB : (g + 1) * TB],
                                pj[:],
                                Ident,
                                bias=b_sb[:, mc_i : mc_i + 1],
                                scale=1.0 / WSC,
                            )
                    for ti in range(4):
                        tc_i = 4 * g + ti
                        pv = ps_sm.tile([P, 512], F32, tag="sm")
                        if FP8_QKV:
                            for i, dc in enumerate(range(0, DC, 2)):
                                nc.tensor.matmul(
                                    pv[:, 0:G],
                                    xTg[g][:, dc : dc + 2, ti * P : (ti + 1) * P],
                                    wv_sb[:, dc : dc + 2, :],
                                    start=(i == 0),
                                    stop=(dc == DC - 2),
                                    perf_mode=DR,
                                )
                        else:
                            for dc in range(DC):
                                nc.tensor.matmul(
                                    pv[:, 0:G],
                                    xTg[g][:, dc, ti * P : (ti + 1) * P],
                                    wv_sb[:, dc, :],
                                    start=(dc == 0),
                                    stop=(dc == DC - 1),
                                )
                        # v = pv/WSC + bv, fp8 store into padded head blocks
                        nc.vector.scalar_tensor_tensor(
                            v_sb[:, tc_i, :, 0:HD],
                            pv[:, 0:G].rearrange("p (h c) -> p h c", c=HD),
                            1.0 / WSC,
                            bv_bc[:].rearrange("p (h c) -> p h c", c=HD),
                            MULT,
                            ADD,
                        )

                # ------ attention, emission-interleaved with QKV blocks ------
                nc.sync.dma_start(
                    wo_sb[:], wo_d.rearrange("(dc p) n -> p dc n", p=P)
                )
                nc.sync.dma_start(bo_row[:], bo_d)
                nc.gpsimd.partition_broadcast(bo_bc[:], bo_row[:])
                work = tc.alloc_tile_pool(name="att", bufs=1)
                pTp = tc.alloc_tile_pool(name="pTp", bufs=8)
                smallp = tc.alloc_tile_pool(name="smallp", bufs=4)
                ctxn = [
                    [
                        work.tile(
                            [HD, TB], BF16, tag=f"ctxn{h}_{q}", name=f"ctxn{h}_{q}"
                        )
                        for q in range(NQT)
                    ]
                    for h in range(NH_LOC)
                ]
                # reciprocal of denominator lives at row 64; other rows stay 0
                # so the broadcast matmul (ones at row 64) reads no garbage
                rdenX = [
                    work.tile([P, 512], F32, tag=f"rdenX{i}", name=f"rdenX{i}")
                    for i in range(2)
                ]
                for i in range(2):
                    nc.vector.tensor_copy(rdenX[i][:], zeros_f[:])
                c_ps_of = {}

                def emit_scores(pair, ch):
                    qt, k0, k1 = ch
                    p_tiles = {}
                    for kc in range(k0, k1):
                        j = kc - 4 * qt
                        coff = max(0, j) * P
                        if kc % 2 == 0:
                            pT = pTp.tile([P, 2, 2, TB], DP, tag="pT")
                            p_tiles[kc] = pT
                        else:
                            pT = p_tiles[kc - 1]
                        s_ps = ps_mm.tile([P, 2, 512], F32, tag="mm")
                        for h01 in range(2):
                            pb = h01 * HD
                            if FP8_SC:
                                nc.tensor.matmul(
                                    s_ps[:, h01, coff:512],
                                    kT[pb : pb + HD, pair, :, kc * P : (kc + 1) * P],
                                    qT[
                                        pb : pb + HD,
                                        pair,
                                        :,
                                        qt * TB + coff : (qt + 1) * TB,
                                    ],
                                    start=True,
                                    stop=True,
                                    perf_mode=DR,
                                )
                            else:
                                nc.tensor.matmul(
                                    s_ps[:, h01, coff:512],
                                    kT[pb : pb + HD, pair, 0, kc * P : (kc + 1) * P],
                                    qT[
                                        pb : pb + HD,
                                        pair,
                                        0,
                                        qt * TB + coff : (qt + 1) * TB,
                                    ],
                                    start=True,
                                    stop=True,
                                )
                        nc.scalar.activation(
                            pT[:, kc % 2, :, coff:512],
                            s_ps[:, :, coff:512],
                            Exp,
                            scale=0.125,
                        )
                        if j >= 0:
                            nc.gpsimd.tensor_tensor(
                                pT[:, kc % 2, :, coff : coff + P],
                                pT[:, kc % 2, :, coff : coff + P],
                                tri_p[:],
                                MULT,
                            )
                    return p_tiles

                def emit_ctx(pair, ch, p_tiles):
                    qt, k0, k1 = ch
                    nkc = 4 * qt + 4
                    if k0 == 0:
                        c_ps_of[pair, qt] = [
                            ps_ctx.tile([P, 512], F32, tag="ctx", name=f"cps{h01}")
                            for h01 in range(2)
                        ]
                    c_ps = c_ps_of[pair, qt]
                    for kc in range(k0, k1):
                        j = kc - 4 * qt
                        pT = p_tiles[kc - kc % 2]
                        coff = max(0, j) * P
                        for h01 in range(2):
                            h = 2 * pair + h01
                            nc.tensor.matmul(
                                c_ps[h01][0 : HD + 1, coff:512],
                                v_sb[:, kc, h, 0 : HD + 1],
                                pT[:, kc % 2, h01, coff:512],
                                start=(kc == 0),
                                stop=(kc == nkc - 1),
                            )
                def emit_norm(pair, qt):
                    # normalize by 1/den (den = ones-row sums at psum row 64),
                    # broadcast across partitions via a row-64 matmul. The two
                    # heads' reciprocals run on different engines (DVE and
                    # ACT Ln->Exp) so they don't serialize.
                    c_ps = c_ps_of[pair, qt]
                    for h01 in range(2):
                        rX = rdenX[h01]
                        with nc.allow_low_precision(reason="den recip"):
                            nc.vector.reciprocal(
                                rX[64:65, :], c_ps[h01][64:65, :]
                            )
                    for h01 in range(2):
                        h = 2 * pair + h01
                        b_ps = ps_sm.tile([P, 512], F32, tag="sm", name="bps")
                        nc.tensor.matmul(
                            b_ps[0:HD, :],
                            zrow_f[:, 0:HD],
                            rdenX[h01][:],
                            start=True,
                            stop=True,
                        )
                        bb = smallp.tile([HD, 512], F32, tag="bb")
                        nc.scalar.copy(bb[:], b_ps[0:HD, :])
                        nc.vector.tensor_tensor(
                            ctxn[h][qt][:, :],
                            c_ps[h01][0:HD, :],
                            bb[:],
                            MULT,
                        )
                    # A2A sends for this (pair, qt): destination block qt,
                    # duplicated across batch halves
                    for sh in (qt, qt + 4):
                        for h01 in range(2):
                            h = 2 * pair + h01
                            nc.sync.dma_start(
                                a2a_in[pair][
                                    sh * P + h01 * HD : sh * P + (h01 + 1) * HD,
                                    :,
                                ],
                                ctxn[h][qt][:, :],
                            )

                def qt_chunks(qt):
                    return [
                        (qt, kcb, min(kcb + 8, 4 * qt + 4))
                        for kcb in range(0, 4 * qt + 4, 8)
                    ]

                # software pipeline: emit scores(i+1) before ctx(i) so the PE
                # queue never blocks the ACT exp stream behind ctx matmuls;
                # pair-0 attention interleaves with the QKV q-block loop
                pend = []
                norms = []

                def push(pair, ch):
                    tiles = emit_scores(pair, ch)
                    # norms lag one more chunk than ctx so the reciprocal
                    # chain never blocks the PE queue at the bcast matmul
                    while norms:
                        emit_norm(*norms.pop(0))
                    if pend:
                        p2, c2, t2 = pend.pop()
                        emit_ctx(p2, c2, t2)
                        if c2[2] == 4 * c2[0] + 4:
                            norms.append((p2, c2[0]))
                    pend.append((pair, ch, tiles))

                def flush():
                    p2, c2, t2 = pend.pop()
                    emit_ctx(p2, c2, t2)
                    if c2[2] == 4 * c2[0] + 4:
                        norms.append((p2, c2[0]))
                    while norms:
                        emit_norm(*norms.pop(0))

                for g in range(NQT):
                    emit_qkv(g)
                    for ch in qt_chunks(g):
                        push(0, ch)
                flush()
                if coll:
                    nc.gpsimd.collective_compute(
                        "AllToAll",
                        mybir.AluOpType.bypass,
                        ins=[a2a_in[0].opt()],
                        outs=[a2a_out[0].opt()],
                        replica_groups=[list(range(NC))],
                    )
                for qt in range(NQT):
                    for ch in qt_chunks(qt):
                        push(1, ch)
                flush()
                if coll:
                    nc.gpsimd.collective_compute(
                        "AllToAll",
                        mybir.AluOpType.bypass,
                        ins=[a2a_in[1].opt()],
                        outs=[a2a_out[1].opt()],
                        replica_groups=[list(range(NC))],
                    )

                smallp.release()
                pTp.release()
                work.release()

            # ---------- receive + output projection ----------
            outp = tc.alloc_tile_pool(name="outp", bufs=1)
            gsrc = a2a_out if coll else a2a_in
            # cx[p, pr, j, t]: sender j's pair-pr dims for my token block
            cx = outp.tile([P, 2, NC, TB], BF16, tag="cx")
            tmp = outp.tile([P, NQT, TB], BF16, tag="seltmp")
            # ctxf[p, pr, g, t]: global dim chunk dc = 2*g + pr
            ctxf = outp.tile([P, 2, NQT, TB], BF16, tag="ctxf")

            with tc.tile_pool(name="out_pool", bufs=3) as out_pool:
                o_parts = [
                    outp.tile([P, 512], F32, tag=f"opart{u}", name=f"opart{u}")
                    for u in range(8)
                ]
                for pr in range(2):
                    nc.sync.dma_start(
                        cx[:, pr],
                        gsrc[pr][:].rearrange("(j q) t -> q j t", q=P),
                    )
                # batch select: ctxf = lo*bsel[0] + hi*bsel[1]
                nc.vector.tensor_scalar(
                    tmp[:], cx[:, 0, 0:4], bsel_sb[:, 0:1], None, MULT
                )
                nc.vector.scalar_tensor_tensor(
                    ctxf[:, 0], cx[:, 0, 4:8], bsel_sb[:, 1:2], tmp[:], MULT, ADD
                )
                # pair-0 (even) chunks first: overlap with A2A of pair 1
                for u in range(8):
                    tc_i, nt = u // 2, u % 2
                    po = ps_mm.tile([P, 2, 512], F32, tag="mm")
                    for i, g in enumerate(range(NQT)):
                        nc.tensor.matmul(
                            po[:, 0, :],
                            ctxf[:, 0, g, tc_i * P : (tc_i + 1) * P],
                            wo_sb[:, 2 * g, nt * 512 : (nt + 1) * 512],
                            start=(i == 0),
                            stop=(i == NQT - 1),
                        )
                    nc.vector.tensor_tensor(
                        o_parts[u][:],
                        po[:, 0, :],
                        bo_bc[:, nt * 512 : (nt + 1) * 512],
                        ADD,
                    )
                nc.vector.tensor_scalar(
                    tmp[:], cx[:, 1, 0:4], bsel_sb[:, 0:1], None, MULT
                )
                nc.vector.scalar_tensor_tensor(
                    ctxf[:, 1], cx[:, 1, 4:8], bsel_sb[:, 1:2], tmp[:], MULT, ADD
                )
                for u in range(8):
                    tc_i, nt = u // 2, u % 2
                    po = ps_mm.tile([P, 2, 512], F32, tag="mm")
                    for i, g in enumerate(range(NQT)):
                        nc.tensor.matmul(
                            po[:, 0, :],
                            ctxf[:, 1, g, tc_i * P : (tc_i + 1) * P],
                            wo_sb[:, 2 * g + 1, nt * 512 : (nt + 1) * 512],
                            start=(i == 0),
                            stop=(i == NQT - 1),
                        )
                    o_sb = out_pool.tile([P, 512], F32, tag="osb")
                    nc.vector.tensor_tensor(
                        o_sb[:], po[:, 0, :], o_parts[u][:], ADD
                    )
                    nc.sync.dma_start(
                        y_d[
                            tc_i * P : (tc_i + 1) * P,
                            nt * 512 : (nt + 1) * 512,
                        ],
                        o_sb[:],
                    )

            outp.release()

    nc.compile()
    return nc


_NC_CACHE = {}


def _get_nc():
    if "nc" not in _NC_CACHE:
        _NC_CACHE["nc"] = build()
    return _NC_CACHE["nc"]


def _make_in_maps(x, Wq, bq, Wk, bk, Wv, bv, Wo, bo):
    x = np.asarray(x, np.float32)
    Wq, Wk, Wv, Wo = (np.asarray(a, np.float32) for a in (Wq, Wk, Wv, Wo))
    bq, bk, bv, bo = (np.asarray(a, np.float32) for a in (bq, bk, bv, bo))
    np_dx = NP_F8 if FP8_QKV else NP_BF16
    wo_b = np.ascontiguousarray(Wo).astype(NP_BF16)
    in_maps = []
    for c in range(NC):
        b, g = c // 4, c % 4
        sl = slice(g * G, (g + 1) * G)
        bsel = np.tile(
            np.array([1.0 - b, float(b)], np.float32).reshape(1, 2), (P, 1)
        )
        in_maps.append(
            {
                "xt": np.ascontiguousarray(x[b].T).astype(np_dx),
                "wq": np.ascontiguousarray(Wq[:, sl] * WSC).astype(np_dx),
                "wk": np.ascontiguousarray(Wk[:, sl] * WSC).astype(np_dx),
                "wv": np.ascontiguousarray(Wv[:, sl] * WSC).astype(np_dx),
                "bq": np.ascontiguousarray(bq[sl].reshape(MC, P).T),
                "bk": np.ascontiguousarray(bk[sl].reshape(MC, P).T),
                "bv": np.ascontiguousarray(bv[sl].reshape(1, G)),
                "wo": wo_b,
                "bo": np.ascontiguousarray(bo.reshape(1, D)),
                "bsel": np.ascontiguousarray(bsel),
            }
        )
    return in_maps


def run(inputs, trace=False, tmpdir=None):
    """Run on 8 cores; returns (output [2,2048,1024], BassKernelResults)."""
    if trace:
        _install_ntff_hook()
    nc = _get_nc()
    in_maps = _make_in_maps(**inputs)
    res = bass_utils.run_bass_kernel_spmd(
        nc, in_maps, core_ids=list(range(NC)), trace=trace, tmpdir=tmpdir
    )
    out = np.empty((B, S, D), np.float32)
    for c in range(NC):
        b, g = c // 4, c % 4
        out[b, g * TB : (g + 1) * TB, :] = res.results[c]["y"]
    return out, res


def kernel(**inputs) -> np.ndarray:
    out, _ = run(inputs, trace=False)
    return out
